# revision 1
# baseline (speedup 1.0000x reference)
"""Trainium2 Bass kernel for an autoregressive LSTM (inference scan).

Model (per reference):
    h0 = c0 = concat([features, features], 1)      # [B, 1024]
    x0 = 0                                         # [B, 1]
    for t in range(128):
        z = x @ kernel + h @ R + bias              # [B, 4096]
        i, f, g, o = sigmoid/sigmoid/tanh/sigmoid of z quarters
        c = f*c + i*g ; h = o*tanh(c)
        pred = h @ dense_w + dense_b               # [B, 1]  (next x)
    out = stack(preds)                             # [B, 128, 1]

Strategy:
  - Data-parallel over batch: 8 cores x 512 rows, weights replicated,
    no collectives. Each core runs the full 128-step scan.
  - Algebraic fold: x_{t+1} = h_t @ dense_w + dense_b, so
        z_{t+1} = h_t @ (R + dense_w x kernel) + (bias + dense_b*kernel)
    i.e. the input-kernel term folds into modified recurrent weights R'
    and bias b'. Step 0 (x=0) is corrected with a single rank-1 matmul
    subtracting (h0 @ dense_w + dense_b) x kernel.
  - Transposed state layout: h,c stored as [u, b] so the per-step matmul
    zT = R'.T @ hT needs no transposes anywhere in the loop, and bias
    lands on the partition axis (free ScalarE bias-add during the gate
    activation).
  - bf16 weights/h (fp32 PSUM accumulation, fp32 cell state c).
"""

import os
import sys

sys.path.insert(0, "/opt/trn_rl_repo")

import numpy as np

import concourse.bass as bass
import concourse.tile as tile
from concourse import bacc, mybir
from concourse.bass_utils import run_bass_kernel_spmd
from concourse.masks import make_identity

B = 4096          # global batch
FEAT = 512        # feature dim (= UNITS // 2)
U = 1024          # LSTM units
J = 4 * U         # gate width
T_STEPS = 128
N_CORES = 8
BL = B // N_CORES  # 512 batch rows per core
KC = U // 128      # 8 contraction chunks
JT = J // 128      # 32 output-row tiles (transposed layout)

F32 = mybir.dt.float32
BF16 = mybir.dt.bfloat16
AF = mybir.ActivationFunctionType
OP = mybir.AluOpType


def build_program(t_steps: int = T_STEPS):
    nc = bacc.Bacc(None, target_bir_lowering=False)

    feat = nc.declare_dram_parameter("feat", [BL, FEAT], F32, isOutput=False)
    rk = nc.declare_dram_parameter("rk", [U, J], F32, isOutput=False)
    kern = nc.declare_dram_parameter("kern", [1, J], F32, isOutput=False)
    bias_d = nc.declare_dram_parameter("bias", [J], F32, isOutput=False)
    dw = nc.declare_dram_parameter("dw", [U, 1], F32, isOutput=False)
    db = nc.declare_dram_parameter("db", [1], F32, isOutput=False)
    # [t, b] layout on device; host transposes to [b, t, 1].
    out = nc.declare_dram_parameter("out", [t_steps, BL], F32, isOutput=True)

    with tile.TileContext(nc) as tc:
        with (
            tc.tile_pool(name="persist", bufs=1) as persist,
            tc.tile_pool(name="zpsum", bufs=6, space="PSUM") as zpsum,
            tc.tile_pool(name="ppsum", bufs=2, space="PSUM") as ppsum,
        ):
            # Persistent SBUF state
            Rp = persist.tile([128, KC, J], BF16, tag="rp")       # R' (folded)
            hA = persist.tile([128, KC, BL], BF16, tag="ha")      # hT ping
            hB = persist.tile([128, KC, BL], BF16, tag="hb")      # hT pong
            cT = persist.tile([128, KC, BL], F32, tag="c")        # cell state
            biasT = persist.tile([128, JT], F32, tag="biast")     # b' transposed
            negk = persist.tile([1, J], BF16, tag="negk")         # -kernel row
            dws = persist.tile([128, KC], BF16, tag="dws")        # dense_w chunks
            db_sb = persist.tile([1, 1], F32, tag="dbsb")
            p0_sb = persist.tile([1, BL], BF16, tag="p0")         # step-0 corr
            zb = persist.tile([128, 1], F32, tag="zb")            # zero bias

            hbufs = [hA, hB]

            # ---------------- setup ----------------
            with (
                tc.tile_pool(name="setup1", bufs=1) as setup1,
                tc.tile_pool(name="setup2", bufs=2) as setup2,
            ):
                nc.vector.memset(zb[:], 0.0)

                ident = setup1.tile([128, 128], F32, tag="ident")
                make_identity(nc, ident[:])

                # dense_w -> [128, KC] (partition = u within chunk)
                dws_f = setup1.tile([128, KC], F32, tag="dwsf")
                nc.sync.dma_start(
                    out=dws_f[:], in_=dw[:, 0].rearrange("(k p) -> p k", p=128)
                )
                nc.vector.tensor_copy(out=dws[:], in_=dws_f[:])

                # dense_b scalar + broadcast
                nc.sync.dma_start(out=db_sb[:], in_=db[:].to_broadcast((1, 1)))
                dbb = setup1.tile([128, 1], F32, tag="dbb")
                nc.sync.dma_start(out=dbb[:], in_=db[:].to_broadcast((128, 1)))

                # bias' = bias + dense_b * kernel   (transposed [128, JT])
                btmp = setup1.tile([128, JT], F32, tag="btmp")
                ktmp = setup1.tile([128, JT], F32, tag="ktmp")
                nc.sync.dma_start(
                    out=btmp[:], in_=bias_d[:].rearrange("(jt p) -> p jt", p=128)
                )
                nc.sync.dma_start(
                    out=ktmp[:], in_=kern[0, :].rearrange("(jt p) -> p jt", p=128)
                )
                nc.vector.scalar_tensor_tensor(
                    out=biasT[:], in0=ktmp[:], scalar=dbb[:, 0:1], in1=btmp[:],
                    op0=OP.mult, op1=OP.add,
                )

                # -kernel row (for the step-0 rank-1 correction)
                krow = setup1.tile([1, J], F32, tag="krow")
                nc.sync.dma_start(out=krow[:], in_=kern[0:1, :])
                nc.scalar.mul(negk[:], krow[:], -1.0)

                # R' = R + dense_w x kernel, cast to bf16, chunked over u
                kbcast = setup1.tile([128, J], F32, tag="kbcast")
                nc.sync.dma_start(out=kbcast[:], in_=kern[0:1, :].to_broadcast((128, J)))
                for k in range(KC):
                    rstage = setup2.tile([128, J], F32, tag="rstage")
                    nc.sync.dma_start(out=rstage[:], in_=rk[k * 128:(k + 1) * 128, :])
                    nc.vector.scalar_tensor_tensor(
                        out=Rp[:, k, :], in0=kbcast[:], scalar=dws_f[:, k:k + 1],
                        in1=rstage[:], op0=OP.mult, op1=OP.add,
                    )

                # h0 = c0 = concat([features, features]) transposed to [u, b]
                for bi in range(4):
                    fsb = setup2.tile([128, FEAT], F32, tag="fsb")
                    nc.sync.dma_start(out=fsb[:], in_=feat[bi * 128:(bi + 1) * 128, :])
                    for fj in range(4):
                        tp = zpsum.tile([128, 512], F32, tag="zp")
                        nc.tensor.transpose(
                            tp[:, 0:128], fsb[:, fj * 128:(fj + 1) * 128], ident[:]
                        )
                        bs = slice(bi * 128, (bi + 1) * 128)
                        nc.vector.tensor_copy(out=hA[:, fj, bs], in_=tp[:, 0:128])
                        nc.vector.tensor_copy(out=hA[:, fj + 4, bs], in_=tp[:, 0:128])
                        nc.vector.tensor_copy(out=cT[:, fj, bs], in_=tp[:, 0:128])
                        nc.vector.tensor_copy(out=cT[:, fj + 4, bs], in_=tp[:, 0:128])

                # p0' = h0 @ dense_w + dense_b  (the pred h0 WOULD emit; its
                # kernel-term contribution must be subtracted from z at t=0)
                pp0 = ppsum.tile([1, BL], F32, tag="pp")
                for k in range(KC):
                    nc.tensor.matmul(
                        pp0[:], dws[:, k:k + 1], hA[:, k, :],
                        start=(k == 0), stop=(k == KC - 1),
                    )
                nc.scalar.activation(
                    out=p0_sb[:], in_=pp0[:], func=AF.Identity, bias=db_sb[0:1, 0:1]
                )

            # ---------------- scan pools ----------------
            with (
                tc.tile_pool(name="gates", bufs=12) as gates_pool,
                tc.tile_pool(name="tmps", bufs=4) as tmp_pool,
                tc.tile_pool(name="ths", bufs=2) as th_pool,
                tc.tile_pool(name="prows", bufs=2) as prow_pool,
            ):
                GATE_FUNCS = [AF.Sigmoid, AF.Sigmoid, AF.Tanh, AF.Sigmoid]

                for t in range(t_steps):
                    hcur = hbufs[t % 2]
                    hnxt = hbufs[(t + 1) % 2]
                    for k in range(KC):
                        gt = []
                        for g in range(4):
                            jt = g * KC + k
                            zp = zpsum.tile([128, BL], F32, tag="zp")
                            for kk in range(KC):
                                nc.tensor.matmul(
                                    zp[:],
                                    Rp[:, kk, jt * 128:(jt + 1) * 128],
                                    hcur[:, kk, :],
                                    start=(kk == 0),
                                    stop=(kk == KC - 1 and t > 0),
                                )
                            if t == 0:
                                # z0 -= p0' x kernel  (x0 is 0, not pred(h0))
                                nc.tensor.matmul(
                                    zp[:],
                                    negk[0:1, jt * 128:(jt + 1) * 128],
                                    p0_sb[:],
                                    start=False, stop=True,
                                )
                            gtile = gates_pool.tile([128, BL], BF16, tag="gate")
                            nc.scalar.activation(
                                out=gtile[:], in_=zp[:], func=GATE_FUNCS[g],
                                bias=biasT[:, jt:jt + 1],
                            )
                            gt.append(gtile)
                        gi, gf, gg, go = gt
                        ig = tmp_pool.tile([128, BL], F32, tag="tmp")
                        nc.vector.tensor_tensor(ig[:], gi[:], gg[:], OP.mult)
                        fc = tmp_pool.tile([128, BL], F32, tag="tmp")
                        nc.vector.tensor_tensor(fc[:], gf[:], cT[:, k, :], OP.mult)
                        nc.vector.tensor_tensor(cT[:, k, :], ig[:], fc[:], OP.add)
                        th = th_pool.tile([128, BL], BF16, tag="th")
                        nc.scalar.activation(
                            out=th[:], in_=cT[:, k, :], func=AF.Tanh, bias=zb[:, 0:1]
                        )
                        nc.vector.tensor_tensor(hnxt[:, k, :], go[:], th[:], OP.mult)

                    # pred_t = h_{t+1-state} @ dense_w + dense_b -> out[t]
                    pp = ppsum.tile([1, BL], F32, tag="pp")
                    for k in range(KC):
                        nc.tensor.matmul(
                            pp[:], dws[:, k:k + 1], hnxt[:, k, :],
                            start=(k == 0), stop=(k == KC - 1),
                        )
                    prow = prow_pool.tile([1, BL], F32, tag="prow")
                    nc.scalar.activation(
                        out=prow[:], in_=pp[:], func=AF.Identity, bias=db_sb[0:1, 0:1]
                    )
                    nc.sync.dma_start(out=out[t:t + 1, :], in_=prow[:])

    nc.compile()
    return nc


_PROGRAM_CACHE = {}


def run(inputs: dict, t_steps: int = T_STEPS, trace: bool = False):
    """Shard inputs, run the SPMD program on 8 cores, gather full output."""
    if t_steps not in _PROGRAM_CACHE:
        _PROGRAM_CACHE[t_steps] = build_program(t_steps)
    nc = _PROGRAM_CACHE[t_steps]

    feats = np.ascontiguousarray(np.asarray(inputs["features"], dtype=np.float32))
    rk = np.ascontiguousarray(np.asarray(inputs["recurrent_kernel"], dtype=np.float32))
    kern = np.ascontiguousarray(
        np.asarray(inputs["kernel"], dtype=np.float32).reshape(1, J)
    )
    bias = np.ascontiguousarray(np.asarray(inputs["bias"], dtype=np.float32))
    dw = np.ascontiguousarray(np.asarray(inputs["dense_w"], dtype=np.float32))
    db = np.ascontiguousarray(
        np.asarray(inputs["dense_b"], dtype=np.float32).reshape(1)
    )

    in_maps = []
    for i in range(N_CORES):
        in_maps.append({
            "feat": feats[i * BL:(i + 1) * BL],
            "rk": rk,
            "kern": kern,
            "bias": bias,
            "dw": dw,
            "db": db,
        })

    res = run_bass_kernel_spmd(
        nc, in_maps, core_ids=list(range(N_CORES)), trace=trace
    )
    # per-core [t, bl] -> full [B, t, 1]
    outs = [np.asarray(res.results[i]["out"]) for i in range(N_CORES)]
    full = np.concatenate([o.T for o in outs], axis=0)[:, :, None]
    return full.astype(np.float32), res


def kernel(**inputs) -> np.ndarray:
    out, _ = run(inputs, t_steps=T_STEPS, trace=False)
    return out


if __name__ == "__main__":
    rng = np.random.default_rng(0)
    inputs = {
        "features": rng.standard_normal((B, FEAT), dtype=np.float32),
        "kernel": rng.standard_normal((1, J), dtype=np.float32) * 0.02,
        "recurrent_kernel": rng.standard_normal((U, J), dtype=np.float32) * 0.02,
        "bias": np.zeros((J,), dtype=np.float32),
        "dense_w": rng.standard_normal((U, 1), dtype=np.float32) * 0.02,
        "dense_b": np.zeros((1,), dtype=np.float32),
    }
    out, _ = run(inputs, t_steps=2)
    print(out.shape, out[:2, :2, 0])



# revision 3
# speedup vs baseline: 1.3333x; 1.3333x over previous
"""Trainium2 Bass kernel for an autoregressive LSTM (inference scan).

Model (per reference):
    h0 = c0 = concat([features, features], 1)      # [B, 1024]
    x0 = 0                                         # [B, 1]
    for t in range(128):
        z = x @ kernel + h @ R + bias              # [B, 4096]
        i, f, g, o = sigmoid/sigmoid/tanh/sigmoid of z quarters
        c = f*c + i*g ; h = o*tanh(c)
        pred = h @ dense_w + dense_b               # [B, 1]  (next x)
    out = stack(preds)                             # [B, 128, 1]

Strategy:
  - Data-parallel over batch: 8 cores x 512 rows, weights replicated,
    no collectives. Each core runs the full 128-step scan.
  - Algebraic fold: x_{t+1} = h_t @ dense_w + dense_b, so
        z_{t+1} = h_t @ (R + dense_w x kernel) + (bias + dense_b*kernel)
    i.e. the input-kernel term folds into modified recurrent weights R'
    and bias b'. Step 0 (x=0) is corrected with a single rank-1 matmul
    subtracting (h0 @ dense_w + dense_b) x kernel.
  - Transposed state layout: h,c stored as [u, b] so the per-step matmul
    zT = R'.T @ hT needs no transposes anywhere in the loop, and bias
    lands on the partition axis (free ScalarE bias-add during the gate
    activation).
  - Mixed per-gate precision: the i/f/o (sigmoid) gates tolerate fp8
    operands (sigmoid' <= 1/4 damps quantization noise), so their z
    columns are computed with fp8e4m3 DoubleRow matmuls (256 contraction
    rows per instruction = 2x bf16 MAC rate). The g (tanh) gate is the
    error-critical path and stays fp16, as does the pred head. Weights
    are pre-scaled by 2^10 and h by 2^5 to sit in e4m3's normal range;
    the 2^-15 is folded into the gate activation's scale operand (free).
    fp32 PSUM accumulation; fp32 cell state c; fp16 gates/h.
"""

import os
import sys

sys.path.insert(0, "/opt/trn_rl_repo")

import numpy as np

import concourse.bass as bass
import concourse.tile as tile
from concourse import bacc, mybir
from concourse.bass_utils import run_bass_kernel_spmd
from concourse.masks import make_identity

B = 4096          # global batch
FEAT = 512        # feature dim (= UNITS // 2)
U = 1024          # LSTM units
J = 4 * U         # gate width
T_STEPS = 128
N_CORES = 8
BL = B // N_CORES  # 512 batch rows per core
KC = U // 128      # 8 contraction chunks of 128
KK2 = U // 256     # 4 DoubleRow super-chunks of 256
JT = J // 128      # 32 output-row tiles (transposed layout)

SW = 1024.0        # fp8 weight pre-scale (2^10)
SH = 32.0          # fp8 h pre-scale (2^5)
INV_S = 1.0 / (SW * SH)

F32 = mybir.dt.float32
FP16 = mybir.dt.float16
FP8 = mybir.dt.float8e4
AF = mybir.ActivationFunctionType
OP = mybir.AluOpType
PM = mybir.MatmulPerfMode

# gate order (i, f, g, o); g (index 2) runs in fp16, the rest in fp8.
FP8_GATES = [0, 1, 3]
GPOS = {0: 0, 1: 1, 3: 2}  # packing position inside the fp8 weight tile


def build_program(t_steps: int = T_STEPS):
    nc = bacc.Bacc(None, target_bir_lowering=False)

    feat = nc.declare_dram_parameter("feat", [BL, FEAT], F32, isOutput=False)
    rk = nc.declare_dram_parameter("rk", [U, J], F32, isOutput=False)
    kern = nc.declare_dram_parameter("kern", [1, J], F32, isOutput=False)
    bias_d = nc.declare_dram_parameter("bias", [J], F32, isOutput=False)
    dw = nc.declare_dram_parameter("dw", [U, 1], F32, isOutput=False)
    db = nc.declare_dram_parameter("db", [1], F32, isOutput=False)
    # [t, b] layout on device; host transposes to [b, t, 1].
    out = nc.declare_dram_parameter("out", [t_steps, BL], F32, isOutput=True)

    with tile.TileContext(nc) as tc:
        with (
            tc.tile_pool(name="persist", bufs=1) as persist,
            tc.tile_pool(name="zpsum", bufs=6, space="PSUM") as zpsum,
            tc.tile_pool(name="ppsum", bufs=2, space="PSUM") as ppsum,
        ):
            # Persistent SBUF state
            R8 = persist.tile([128, KK2, 2, 3 * U], FP8, tag="r8")   # i,f,o cols
            Rg = persist.tile([128, KC, U], FP16, tag="rg")          # g cols
            hfA = persist.tile([128, KC, BL], FP16, tag="hfa")       # h ping
            hfB = persist.tile([128, KC, BL], FP16, tag="hfb")       # h pong
            h8A = persist.tile([128, KK2, 2, BL], FP8, tag="h8a")    # h*2^5 ping
            h8B = persist.tile([128, KK2, 2, BL], FP8, tag="h8b")    # h*2^5 pong
            cT = persist.tile([128, KC, BL], F32, tag="c")           # cell state
            biasT = persist.tile([128, JT], F32, tag="biast")        # b' transposed
            negk_s = persist.tile([1, 3 * U], FP16, tag="negks")     # -k*2^15, ifo
            negk_g = persist.tile([1, U], FP16, tag="negkg")         # -k, g cols
            dws = persist.tile([128, KC], FP16, tag="dws")           # dense_w
            db_sb = persist.tile([1, 1], F32, tag="dbsb")
            p0_sb = persist.tile([1, BL], FP16, tag="p0")            # step-0 corr
            zb = persist.tile([128, 1], F32, tag="zb")               # zero bias

            hfbufs = [hfA, hfB]
            h8bufs = [h8A, h8B]

            # ---------------- setup ----------------
            with (
                tc.tile_pool(name="setup1", bufs=1) as setup1,
                tc.tile_pool(name="setup2", bufs=2) as setup2,
            ):
                nc.vector.memset(zb[:], 0.0)

                ident = setup1.tile([128, 128], F32, tag="ident")
                make_identity(nc, ident[:])

                # dense_w -> [128, KC] (partition = u within chunk)
                dws_f = setup1.tile([128, KC], F32, tag="dwsf")
                nc.sync.dma_start(
                    out=dws_f[:], in_=dw[:, 0].rearrange("(k p) -> p k", p=128)
                )
                nc.vector.tensor_copy(out=dws[:], in_=dws_f[:])

                # dense_b scalar + broadcast
                nc.sync.dma_start(out=db_sb[:], in_=db[:].to_broadcast((1, 1)))
                dbb = setup1.tile([128, 1], F32, tag="dbb")
                nc.sync.dma_start(out=dbb[:], in_=db[:].to_broadcast((128, 1)))

                # bias' = bias + dense_b * kernel   (transposed [128, JT])
                btmp = setup1.tile([128, JT], F32, tag="btmp")
                ktmp = setup1.tile([128, JT], F32, tag="ktmp")
                nc.sync.dma_start(
                    out=btmp[:], in_=bias_d[:].rearrange("(jt p) -> p jt", p=128)
                )
                nc.sync.dma_start(
                    out=ktmp[:], in_=kern[0, :].rearrange("(jt p) -> p jt", p=128)
                )
                nc.vector.scalar_tensor_tensor(
                    out=biasT[:], in0=ktmp[:], scalar=dbb[:, 0:1], in1=btmp[:],
                    op0=OP.mult, op1=OP.add,
                )

                # -kernel rows for the step-0 rank-1 correction:
                # scaled by 2^15 for the fp8 gates (their PSUM holds z*2^15),
                # plain for the fp16 g gate.
                krow = setup1.tile([1, J], F32, tag="krow")
                nc.sync.dma_start(out=krow[:], in_=kern[0:1, :])
                for g in FP8_GATES:
                    nc.vector.tensor_scalar_mul(
                        negk_s[0:1, GPOS[g] * U:(GPOS[g] + 1) * U],
                        krow[0:1, g * U:(g + 1) * U],
                        -SW * SH,
                    )
                nc.vector.tensor_scalar_mul(
                    negk_g[0:1, :], krow[0:1, 2 * U:3 * U], -1.0
                )

                # R' = R + dense_w x kernel; split per-gate into fp8 (scaled)
                # and fp16 tiles, chunked over u.
                kbcast = setup1.tile([128, J], F32, tag="kbcast")
                nc.sync.dma_start(out=kbcast[:], in_=kern[0:1, :].to_broadcast((128, J)))
                for k in range(KC):
                    rstage = setup2.tile([128, J], F32, tag="rstage")
                    nc.sync.dma_start(out=rstage[:], in_=rk[k * 128:(k + 1) * 128, :])
                    rfold = setup2.tile([128, J], F32, tag="rfold")
                    nc.vector.scalar_tensor_tensor(
                        out=rfold[:], in0=kbcast[:], scalar=dws_f[:, k:k + 1],
                        in1=rstage[:], op0=OP.mult, op1=OP.add,
                    )
                    nc.vector.tensor_copy(
                        out=Rg[:, k, :], in_=rfold[:, 2 * U:3 * U]
                    )
                    for g in FP8_GATES:
                        nc.vector.tensor_scalar_mul(
                            R8[:, k // 2, k % 2, GPOS[g] * U:(GPOS[g] + 1) * U],
                            rfold[:, g * U:(g + 1) * U],
                            SW,
                        )

                # h0 = c0 = concat([features, features]) transposed to [u, b]
                for bi in range(4):
                    fsb = setup2.tile([128, FEAT], F32, tag="fsb")
                    nc.sync.dma_start(out=fsb[:], in_=feat[bi * 128:(bi + 1) * 128, :])
                    for fj in range(4):
                        tp = zpsum.tile([128, 512], F32, tag="zp")
                        nc.tensor.transpose(
                            tp[:, 0:128], fsb[:, fj * 128:(fj + 1) * 128], ident[:]
                        )
                        bs = slice(bi * 128, (bi + 1) * 128)
                        nc.vector.tensor_copy(out=hfA[:, fj, bs], in_=tp[:, 0:128])
                        nc.vector.tensor_copy(out=hfA[:, fj + 4, bs], in_=tp[:, 0:128])
                        nc.vector.tensor_copy(out=cT[:, fj, bs], in_=tp[:, 0:128])
                        nc.vector.tensor_copy(out=cT[:, fj + 4, bs], in_=tp[:, 0:128])

                # fp8 (scaled) view of h0
                for k in range(KC):
                    nc.vector.tensor_scalar_mul(
                        h8A[:, k // 2, k % 2, :], hfA[:, k, :], SH
                    )

                # p0' = h0 @ dense_w + dense_b  (the pred h0 WOULD emit; its
                # kernel-term contribution must be subtracted from z at t=0)
                pp0 = ppsum.tile([1, BL], F32, tag="pp")
                for k in range(KC):
                    nc.tensor.matmul(
                        pp0[:], dws[:, k:k + 1], hfA[:, k, :],
                        start=(k == 0), stop=(k == KC - 1),
                    )
                nc.scalar.activation(
                    out=p0_sb[:], in_=pp0[:], func=AF.Identity, bias=db_sb[0:1, 0:1]
                )

            # ---------------- scan pools ----------------
            with (
                tc.tile_pool(name="gates", bufs=12) as gates_pool,
                tc.tile_pool(name="tmps", bufs=4) as tmp_pool,
                tc.tile_pool(name="ths", bufs=2) as th_pool,
                tc.tile_pool(name="prows", bufs=2) as prow_pool,
            ):
                GATE_FUNCS = [AF.Sigmoid, AF.Sigmoid, AF.Tanh, AF.Sigmoid]

                for t in range(t_steps):
                    hcurf = hfbufs[t % 2]
                    hcur8 = h8bufs[t % 2]
                    hnxtf = hfbufs[(t + 1) % 2]
                    hnxt8 = h8bufs[(t + 1) % 2]
                    for k in range(KC):
                        gt = []
                        for g in range(4):
                            jt = g * KC + k
                            zp = zpsum.tile([128, BL], F32, tag="zp")
                            if g in GPOS:  # fp8 DoubleRow path (i, f, o)
                                c0 = GPOS[g] * U + k * 128
                                for kk in range(KK2):
                                    nc.tensor.matmul(
                                        zp[:],
                                        R8[:, kk, :, c0:c0 + 128],
                                        hcur8[:, kk, :, :],
                                        start=(kk == 0),
                                        stop=(kk == KK2 - 1 and t > 0),
                                        perf_mode=PM.DoubleRow,
                                    )
                                if t == 0:
                                    nc.tensor.matmul(
                                        zp[:],
                                        negk_s[0:1, c0:c0 + 128],
                                        p0_sb[:],
                                        start=False, stop=True,
                                    )
                                act_scale = INV_S
                            else:  # fp16 path (g gate)
                                for kk in range(KC):
                                    nc.tensor.matmul(
                                        zp[:],
                                        Rg[:, kk, k * 128:(k + 1) * 128],
                                        hcurf[:, kk, :],
                                        start=(kk == 0),
                                        stop=(kk == KC - 1 and t > 0),
                                    )
                                if t == 0:
                                    nc.tensor.matmul(
                                        zp[:],
                                        negk_g[0:1, k * 128:(k + 1) * 128],
                                        p0_sb[:],
                                        start=False, stop=True,
                                    )
                                act_scale = 1.0
                            gtile = gates_pool.tile([128, BL], FP16, tag="gate")
                            nc.scalar.activation(
                                out=gtile[:], in_=zp[:], func=GATE_FUNCS[g],
                                bias=biasT[:, jt:jt + 1], scale=act_scale,
                            )
                            gt.append(gtile)
                        gi, gf, gg, go = gt
                        ig = tmp_pool.tile([128, BL], FP16, tag="ig")
                        nc.vector.tensor_tensor(ig[:], gi[:], gg[:], OP.mult)
                        fc = tmp_pool.tile([128, BL], F32, tag="fc")
                        nc.vector.tensor_tensor(fc[:], gf[:], cT[:, k, :], OP.mult)
                        nc.vector.tensor_tensor(cT[:, k, :], ig[:], fc[:], OP.add)
                        th = th_pool.tile([128, BL], FP16, tag="th")
                        nc.scalar.activation(
                            out=th[:], in_=cT[:, k, :], func=AF.Tanh, bias=zb[:, 0:1]
                        )
                        nc.vector.tensor_tensor(hnxtf[:, k, :], go[:], th[:], OP.mult)
                        nc.vector.tensor_scalar_mul(
                            hnxt8[:, k // 2, k % 2, :], hnxtf[:, k, :], SH
                        )

                    # pred_t = h_{t+1-state} @ dense_w + dense_b -> out[t]
                    pp = ppsum.tile([1, BL], F32, tag="pp")
                    for k in range(KC):
                        nc.tensor.matmul(
                            pp[:], dws[:, k:k + 1], hnxtf[:, k, :],
                            start=(k == 0), stop=(k == KC - 1),
                        )
                    prow = prow_pool.tile([1, BL], F32, tag="prow")
                    nc.scalar.activation(
                        out=prow[:], in_=pp[:], func=AF.Identity, bias=db_sb[0:1, 0:1]
                    )
                    nc.sync.dma_start(out=out[t:t + 1, :], in_=prow[:])

    nc.compile()
    return nc


_PROGRAM_CACHE = {}


def run(inputs: dict, t_steps: int = T_STEPS, trace: bool = False):
    """Shard inputs, run the SPMD program on 8 cores, gather full output."""
    if t_steps not in _PROGRAM_CACHE:
        _PROGRAM_CACHE[t_steps] = build_program(t_steps)
    nc = _PROGRAM_CACHE[t_steps]

    feats = np.ascontiguousarray(np.asarray(inputs["features"], dtype=np.float32))
    rk = np.ascontiguousarray(np.asarray(inputs["recurrent_kernel"], dtype=np.float32))
    kern = np.ascontiguousarray(
        np.asarray(inputs["kernel"], dtype=np.float32).reshape(1, J)
    )
    bias = np.ascontiguousarray(np.asarray(inputs["bias"], dtype=np.float32))
    dw = np.ascontiguousarray(np.asarray(inputs["dense_w"], dtype=np.float32))
    db = np.ascontiguousarray(
        np.asarray(inputs["dense_b"], dtype=np.float32).reshape(1)
    )

    in_maps = []
    for i in range(N_CORES):
        in_maps.append({
            "feat": feats[i * BL:(i + 1) * BL],
            "rk": rk,
            "kern": kern,
            "bias": bias,
            "dw": dw,
            "db": db,
        })

    res = run_bass_kernel_spmd(
        nc, in_maps, core_ids=list(range(N_CORES)), trace=trace
    )
    # per-core [t, bl] -> full [B, t, 1]
    outs = [np.asarray(res.results[i]["out"]) for i in range(N_CORES)]
    full = np.concatenate([o.T for o in outs], axis=0)[:, :, None]
    return full.astype(np.float32), res


def kernel(**inputs) -> np.ndarray:
    out, _ = run(inputs, t_steps=T_STEPS, trace=False)
    return out


if __name__ == "__main__":
    rng = np.random.default_rng(0)
    inputs = {
        "features": rng.standard_normal((B, FEAT), dtype=np.float32),
        "kernel": rng.standard_normal((1, J), dtype=np.float32) * 0.02,
        "recurrent_kernel": rng.standard_normal((U, J), dtype=np.float32) * 0.02,
        "bias": np.zeros((J,), dtype=np.float32),
        "dense_w": rng.standard_normal((U, 1), dtype=np.float32) * 0.02,
        "dense_b": np.zeros((1,), dtype=np.float32),
    }
    out, _ = run(inputs, t_steps=2)
    print(out.shape, out[:2, :2, 0])


# revision 9
# speedup vs baseline: 1.6861x; 1.2646x over previous
"""Trainium2 Bass kernel for an autoregressive LSTM (inference scan).

Model (per reference):
    h0 = c0 = concat([features, features], 1)      # [B, 1024]
    x0 = 0                                         # [B, 1]
    for t in range(128):
        z = x @ kernel + h @ R + bias              # [B, 4096]
        i, f, g, o = sigmoid/sigmoid/tanh/sigmoid of z quarters
        c = f*c + i*g ; h = o*tanh(c)
        pred = h @ dense_w + dense_b               # [B, 1]  (next x)
    out = stack(preds)                             # [B, 128, 1]

Strategy:
  - Data-parallel over batch: 8 cores x 512 rows, weights replicated,
    no collectives. Each core runs the full 128-step scan.
  - Algebraic fold: x_{t+1} = h_t @ dense_w + dense_b, so
        z_{t+1} = h_t @ (R + dense_w x kernel) + (bias + dense_b*kernel)
    i.e. the input-kernel term folds into modified recurrent weights R'
    and bias b'. Step 0 (x=0) is corrected with a single rank-1 matmul
    subtracting (h0 @ dense_w + dense_b) x kernel.
  - Transposed state layout: h,c stored as [u, b] so the per-step matmul
    zT = R'.T @ hT needs no transposes anywhere in the loop, and bias
    lands on the partition axis (free ScalarE bias-add during the gate
    activation).
  - Mixed per-gate precision: the i/f/o (sigmoid) gates tolerate fp8
    operands (sigmoid' <= 1/4 damps quantization noise), so their z
    columns are computed with fp8e4m3 DoubleRow matmuls (256 contraction
    rows per instruction = 2x bf16 MAC rate). The g (tanh) gate is the
    error-critical path and stays fp16, as does the pred head. Weights
    are pre-scaled by 2^10 and h by 2^5 to sit in e4m3's normal range;
    the 2^-15 is folded into the gate activation's scale operand (free).
    fp32 PSUM accumulation; fp32 cell state c; fp16 gates/h.
"""

import os
import sys

sys.path.insert(0, "/opt/trn_rl_repo")

import numpy as np

import concourse.bass as bass
import concourse.tile as tile
from concourse import bacc, mybir
from concourse.bass_utils import run_bass_kernel_spmd
from concourse.masks import make_identity

B = 4096          # global batch
FEAT = 512        # feature dim (= UNITS // 2)
U = 1024          # LSTM units
J = 4 * U         # gate width
T_STEPS = 128
N_CORES = 8
BL = B // N_CORES  # 512 batch rows per core
KC = U // 128      # 8 contraction chunks of 128
KK2 = U // 256     # 4 DoubleRow super-chunks of 256
JT = J // 128      # 32 output-row tiles (transposed layout)

SW = 1024.0        # fp8 weight pre-scale (2^10)
SH = 32.0          # fp8 h pre-scale (2^5)
INV_S = 1.0 / (SW * SH)

F32 = mybir.dt.float32
BF16 = mybir.dt.bfloat16
FP8 = mybir.dt.float8e4
AF = mybir.ActivationFunctionType
OP = mybir.AluOpType
PM = mybir.MatmulPerfMode

# gate order (i, f, g, o); g (index 2) runs in fp16, the rest in fp8.
FP8_GATES = [0, 1, 3]
GPOS = {0: 0, 1: 1, 3: 2}  # packing position inside the fp8 weight tile


def build_program(t_steps: int = T_STEPS):
    nc = bacc.Bacc(None, target_bir_lowering=False)

    feat = nc.declare_dram_parameter("feat", [BL, FEAT], F32, isOutput=False)
    rk = nc.declare_dram_parameter("rk", [U, J], F32, isOutput=False)
    kern = nc.declare_dram_parameter("kern", [1, J], F32, isOutput=False)
    bias_d = nc.declare_dram_parameter("bias", [J], F32, isOutput=False)
    dw = nc.declare_dram_parameter("dw", [U, 1], F32, isOutput=False)
    db = nc.declare_dram_parameter("db", [1], F32, isOutput=False)
    # [t, b] layout on device; host transposes to [b, t, 1].
    out = nc.declare_dram_parameter("out", [t_steps, BL], F32, isOutput=True)

    with tile.TileContext(nc) as tc:
        with (
            tc.tile_pool(name="persist", bufs=1) as persist,
            tc.tile_pool(name="zpsum", bufs=7, space="PSUM") as zpsum,
            tc.tile_pool(name="ppsum", bufs=1, space="PSUM") as ppsum,
        ):
            # Persistent SBUF state
            R8 = persist.tile([128, KK2, 2, 3 * U], FP8, tag="r8")   # i,f,o cols
            Rg = persist.tile([128, KC, U], BF16, tag="rg")          # g cols
            hfA = persist.tile([128, KC, BL], BF16, tag="hfa")       # h ping
            hfB = persist.tile([128, KC, BL], BF16, tag="hfb")       # h pong
            h8A = persist.tile([128, KK2, 2, BL], FP8, tag="h8a")    # h*2^5 ping
            h8B = persist.tile([128, KK2, 2, BL], FP8, tag="h8b")    # h*2^5 pong
            cT = persist.tile([128, KC, BL], F32, tag="c")           # cell state
            biasT = persist.tile([128, JT], F32, tag="biast")        # b' transposed
            negk_s = persist.tile([1, 3 * U], BF16, tag="negks")     # -k*2^15, ifo
            negk_g = persist.tile([1, U], BF16, tag="negkg")         # -k, g cols
            dws = persist.tile([128, KC], BF16, tag="dws")           # dense_w
            dws32 = persist.tile([128, KC], F32, tag="dws32")        # fp32 copy
            ones = persist.tile([128, 1], BF16, tag="ones")          # reduce lhsT
            db_sb = persist.tile([1, 1], F32, tag="dbsb")
            p0_sb = persist.tile([1, BL], BF16, tag="p0")            # step-0 corr
            zb = persist.tile([128, 1], F32, tag="zb")               # zero bias

            hfbufs = [hfA, hfB]
            h8bufs = [h8A, h8B]

            # ---------------- setup ----------------
            with (
                tc.tile_pool(name="setup1", bufs=1) as setup1,
                tc.tile_pool(name="setup2", bufs=2) as setup2,
            ):
                nc.vector.memset(zb[:], 0.0)
                nc.vector.memset(ones[:], 1.0)

                ident = setup1.tile([128, 128], F32, tag="ident")
                make_identity(nc, ident[:])

                # dense_w -> [128, KC] (partition = u within chunk)
                dws_f = dws32
                nc.sync.dma_start(
                    out=dws_f[:], in_=dw[:, 0].rearrange("(k p) -> p k", p=128)
                )
                nc.vector.tensor_copy(out=dws[:], in_=dws_f[:])

                # dense_b scalar + broadcast
                nc.sync.dma_start(out=db_sb[:], in_=db[:].to_broadcast((1, 1)))
                dbb = setup1.tile([128, 1], F32, tag="dbb")
                nc.sync.dma_start(out=dbb[:], in_=db[:].to_broadcast((128, 1)))

                # bias' = bias + dense_b * kernel   (transposed [128, JT])
                btmp = setup1.tile([128, JT], F32, tag="btmp")
                ktmp = setup1.tile([128, JT], F32, tag="ktmp")
                nc.sync.dma_start(
                    out=btmp[:], in_=bias_d[:].rearrange("(jt p) -> p jt", p=128)
                )
                nc.sync.dma_start(
                    out=ktmp[:], in_=kern[0, :].rearrange("(jt p) -> p jt", p=128)
                )
                nc.vector.scalar_tensor_tensor(
                    out=biasT[:], in0=ktmp[:], scalar=dbb[:, 0:1], in1=btmp[:],
                    op0=OP.mult, op1=OP.add,
                )

                # -kernel rows for the step-0 rank-1 correction:
                # scaled by 2^15 for the fp8 gates (their PSUM holds z*2^15),
                # plain for the fp16 g gate.
                krow = setup1.tile([1, J], F32, tag="krow")
                nc.sync.dma_start(out=krow[:], in_=kern[0:1, :])
                for g in FP8_GATES:
                    nc.vector.tensor_scalar_mul(
                        negk_s[0:1, GPOS[g] * U:(GPOS[g] + 1) * U],
                        krow[0:1, g * U:(g + 1) * U],
                        -SW * SH,
                    )
                nc.vector.tensor_scalar_mul(
                    negk_g[0:1, :], krow[0:1, 2 * U:3 * U], -1.0
                )

                # R' = R + dense_w x kernel; split per-gate into fp8 (scaled)
                # and fp16 tiles, chunked over u.
                kbcast = setup1.tile([128, J], F32, tag="kbcast")
                nc.sync.dma_start(out=kbcast[:], in_=kern[0:1, :].to_broadcast((128, J)))
                for k in range(KC):
                    rstage = setup2.tile([128, J], F32, tag="rstage")
                    nc.sync.dma_start(out=rstage[:], in_=rk[k * 128:(k + 1) * 128, :])
                    rfold = setup2.tile([128, J], F32, tag="rfold")
                    nc.vector.scalar_tensor_tensor(
                        out=rfold[:], in0=kbcast[:], scalar=dws_f[:, k:k + 1],
                        in1=rstage[:], op0=OP.mult, op1=OP.add,
                    )
                    nc.vector.tensor_copy(
                        out=Rg[:, k, :], in_=rfold[:, 2 * U:3 * U]
                    )
                    for g in FP8_GATES:
                        nc.vector.tensor_scalar_mul(
                            R8[:, k // 2, k % 2, GPOS[g] * U:(GPOS[g] + 1) * U],
                            rfold[:, g * U:(g + 1) * U],
                            SW,
                        )

                # h0 = c0 = concat([features, features]) transposed to [u, b]
                for bi in range(4):
                    fsb = setup2.tile([128, FEAT], F32, tag="fsb")
                    nc.sync.dma_start(out=fsb[:], in_=feat[bi * 128:(bi + 1) * 128, :])
                    for fj in range(4):
                        tp = zpsum.tile([128, 512], F32, tag="zp")
                        nc.tensor.transpose(
                            tp[:, 0:128], fsb[:, fj * 128:(fj + 1) * 128], ident[:]
                        )
                        bs = slice(bi * 128, (bi + 1) * 128)
                        nc.vector.tensor_copy(out=hfA[:, fj, bs], in_=tp[:, 0:128])
                        nc.vector.tensor_copy(out=hfA[:, fj + 4, bs], in_=tp[:, 0:128])
                        nc.vector.tensor_copy(out=cT[:, fj, bs], in_=tp[:, 0:128])
                        nc.vector.tensor_copy(out=cT[:, fj + 4, bs], in_=tp[:, 0:128])

                # fp8 (scaled) view of h0
                for k in range(KC):
                    nc.vector.tensor_scalar_mul(
                        h8A[:, k // 2, k % 2, :], hfA[:, k, :], SH
                    )

                # p0' = h0 @ dense_w + dense_b  (the pred h0 WOULD emit; its
                # kernel-term contribution must be subtracted from z at t=0)
                pp0 = ppsum.tile([1, BL], F32, tag="pp")
                for k in range(KC):
                    nc.tensor.matmul(
                        pp0[:], dws[:, k:k + 1], hfA[:, k, :],
                        start=(k == 0), stop=(k == KC - 1),
                    )
                nc.scalar.activation(
                    out=p0_sb[:], in_=pp0[:], func=AF.Identity, bias=db_sb[0:1, 0:1]
                )

            # ---------------- scan pools ----------------
            with (
                tc.tile_pool(name="gates", bufs=12) as gates_pool,
                tc.tile_pool(name="tmps", bufs=4) as tmp_pool,
                tc.tile_pool(name="ths", bufs=2) as th_pool,
                tc.tile_pool(name="accs", bufs=2) as acc_pool,
                tc.tile_pool(name="prows", bufs=2) as prow_pool,
            ):
                GATE_FUNCS = [AF.Sigmoid, AF.Sigmoid, AF.Tanh, AF.Sigmoid]

                for t in range(t_steps):
                    hcurf = hfbufs[t % 2]
                    hcur8 = h8bufs[t % 2]
                    hnxtf = hfbufs[(t + 1) % 2]
                    hnxt8 = h8bufs[(t + 1) % 2]
                    for k in range(KC):
                        gt = []
                        for g in range(4):
                            jt = g * KC + k
                            zp = zpsum.tile([128, BL], F32, tag="zp")
                            if g in GPOS:  # fp8 DoubleRow path (i, f, o)
                                c0 = GPOS[g] * U + k * 128
                                for kk in range(KK2):
                                    nc.tensor.matmul(
                                        zp[:],
                                        R8[:, kk, :, c0:c0 + 128],
                                        hcur8[:, kk, :, :],
                                        start=(kk == 0),
                                        stop=(kk == KK2 - 1 and t > 0),
                                        perf_mode=PM.DoubleRow,
                                    )
                                if t == 0:
                                    nc.tensor.matmul(
                                        zp[:],
                                        negk_s[0:1, c0:c0 + 128],
                                        p0_sb[:],
                                        start=False, stop=True,
                                    )
                                act_scale = INV_S
                            else:  # fp16 path (g gate)
                                for kk in range(KC):
                                    nc.tensor.matmul(
                                        zp[:],
                                        Rg[:, kk, k * 128:(k + 1) * 128],
                                        hcurf[:, kk, :],
                                        start=(kk == 0),
                                        stop=(kk == KC - 1 and t > 0),
                                    )
                                if t == 0:
                                    nc.tensor.matmul(
                                        zp[:],
                                        negk_g[0:1, k * 128:(k + 1) * 128],
                                        p0_sb[:],
                                        start=False, stop=True,
                                    )
                                act_scale = 1.0
                            gtile = gates_pool.tile([128, BL], BF16, tag="gate")
                            nc.scalar.activation(
                                out=gtile[:], in_=zp[:], func=GATE_FUNCS[g],
                                bias=biasT[:, jt:jt + 1], scale=act_scale,
                            )
                            gt.append(gtile)
                        gi, gf, gg, go = gt
                        ig = tmp_pool.tile([128, BL], BF16, tag="ig")
                        nc.vector.tensor_tensor(ig[:], gi[:], gg[:], OP.mult)
                        fc = tmp_pool.tile([128, BL], F32, tag="fc")
                        nc.vector.tensor_tensor(fc[:], gf[:], cT[:, k, :], OP.mult)
                        nc.vector.tensor_tensor(cT[:, k, :], ig[:], fc[:], OP.add)
                        th = th_pool.tile([128, BL], BF16, tag="th")
                        nc.scalar.activation(
                            out=th[:], in_=cT[:, k, :], func=AF.Tanh, bias=zb[:, 0:1]
                        )
                        nc.vector.tensor_tensor(hnxtf[:, k, :], go[:], th[:], OP.mult)
                        nc.vector.tensor_scalar_mul(
                            hnxt8[:, k // 2, k % 2, :], hnxtf[:, k, :], SH
                        )
                        # pred partial: acc_k = dws_k (*) h_k (+ acc_{k-1}),
                        # chipping the dense head off TensorE (DVE chain +
                        # one partition-reduce matmul at the end). fp32
                        # intermediates; the final link rounds to bf16 so
                        # the reduce-matmul runs at full speed.
                        last = k == KC - 1
                        acc = acc_pool.tile(
                            [128, BL], BF16 if last else F32,
                            tag="accb" if last else "accf",
                        )
                        if k == 0:
                            nc.vector.tensor_scalar(
                                acc[:], hnxtf[:, k, :], dws32[:, k:k + 1], None,
                                OP.mult,
                            )
                        else:
                            nc.vector.scalar_tensor_tensor(
                                out=acc[:], in0=hnxtf[:, k, :],
                                scalar=dws32[:, k:k + 1], in1=prev_acc[:],
                                op0=OP.mult, op1=OP.add,
                            )
                        prev_acc = acc

                    # pred_t = h_{t+1-state} @ dense_w + dense_b -> out[t]
                    pp = ppsum.tile([1, BL], F32, tag="pp")
                    nc.tensor.matmul(pp[:], ones[:, 0:1], prev_acc[:])
                    prow = prow_pool.tile([1, BL], F32, tag="prow")
                    nc.vector.tensor_scalar(
                        prow[:], pp[:], db_sb[0:1, 0:1], None, OP.add
                    )
                    nc.sync.dma_start(out=out[t:t + 1, :], in_=prow[:])

    nc.compile()
    return nc


_PROGRAM_CACHE = {}


def run(inputs: dict, t_steps: int = T_STEPS, trace: bool = False):
    """Shard inputs, run the SPMD program on 8 cores, gather full output."""
    if t_steps not in _PROGRAM_CACHE:
        _PROGRAM_CACHE[t_steps] = build_program(t_steps)
    nc = _PROGRAM_CACHE[t_steps]

    feats = np.ascontiguousarray(np.asarray(inputs["features"], dtype=np.float32))
    rk = np.ascontiguousarray(np.asarray(inputs["recurrent_kernel"], dtype=np.float32))
    kern = np.ascontiguousarray(
        np.asarray(inputs["kernel"], dtype=np.float32).reshape(1, J)
    )
    bias = np.ascontiguousarray(np.asarray(inputs["bias"], dtype=np.float32))
    dw = np.ascontiguousarray(np.asarray(inputs["dense_w"], dtype=np.float32))
    db = np.ascontiguousarray(
        np.asarray(inputs["dense_b"], dtype=np.float32).reshape(1)
    )

    in_maps = []
    for i in range(N_CORES):
        in_maps.append({
            "feat": feats[i * BL:(i + 1) * BL],
            "rk": rk,
            "kern": kern,
            "bias": bias,
            "dw": dw,
            "db": db,
        })

    res = run_bass_kernel_spmd(
        nc, in_maps, core_ids=list(range(N_CORES)), trace=trace
    )
    # per-core [t, bl] -> full [B, t, 1]
    outs = [np.asarray(res.results[i]["out"]) for i in range(N_CORES)]
    full = np.concatenate([o.T for o in outs], axis=0)[:, :, None]
    return full.astype(np.float32), res


def kernel(**inputs) -> np.ndarray:
    out, _ = run(inputs, t_steps=T_STEPS, trace=False)
    return out


if __name__ == "__main__":
    rng = np.random.default_rng(0)
    inputs = {
        "features": rng.standard_normal((B, FEAT), dtype=np.float32),
        "kernel": rng.standard_normal((1, J), dtype=np.float32) * 0.02,
        "recurrent_kernel": rng.standard_normal((U, J), dtype=np.float32) * 0.02,
        "bias": np.zeros((J,), dtype=np.float32),
        "dense_w": rng.standard_normal((U, 1), dtype=np.float32) * 0.02,
        "dense_b": np.zeros((1,), dtype=np.float32),
    }
    out, _ = run(inputs, t_steps=2)
    print(out.shape, out[:2, :2, 0])
